# revision 6
# baseline (speedup 1.0000x reference)
"""GridRNN Trainium2 kernel.

Problem: 2-D grid RNN, B=4, S=T=128, H=256, D=3 depths.
  hx[d][b,i,j] = tanh(xin @ Wx_ih[d].T + bx_ih[d] + hx[d][b,i-1,(j-1)%T] @ Wx_hh[d].T + bx_hh[d])
  hy[d][b,i,j] = tanh(yin @ Wy_ih[d].T + by_ih[d] + hy[d][b,i,j-1]     @ Wy_hh[d].T + by_hh[d])
  (xin/yin = src/trg broadcast at d=0, previous depth's hx/hy for d>0)
  out = stack([hx[D-1], hy[D-1]], axis=-2)   # [B,S,T,2,H]

Key structure: the x-chain and y-chain never mix across depths -> 8 cores =
4 batches x 2 chains.  The x-chain's diagonal dependence hx[i-1,(j-1)%T] is
removed by shearing: u_i[c] = hx[i,(i+c)%T] turns it into a plain carry
u_{i-1}[c], identical in form to the y-chain.  One SPMD program runs on all
8 cores; only the input data (weights, seed-derived pre0) differs per core.
The host unshears the x outputs and transposes the y outputs.

Matmuls run in fp16 (1 cycle/row on the PE vs 4 for fp32); PSUM accumulates
fp32 and tanh reads the fp32 PSUM, so only operand rounding (~5e-4) enters
per step.  Depth-0's input term depends only on the step index, so it is
precomputed on the HOST and folded into the d0 tanh bias.

ACT-instruction economy: the d1/d2 biases are injected into PSUM by a single
rank-4 matmul (lhsT = 4 bias rows, rhs = block-identity), so the d1+d2 tanh
fuses into ONE [128,512] ACT per tick whose only upstream writer engine is
the PE (this walrus build allows only ONE sync-wait per hardware
instruction, so every PSUM byte an ACT reads must be last-written by the
same engine).  d2(t-2) and d1(t-1) land in the same slot of a big linear
SBUF tile, making the fused ACT output contiguous; the d2 plane leaves in a
few large chunked DMAs.  d0 keeps per-m ACTs with the pre0 column bias.
"""

import numpy as np

import concourse.bass as bass
import concourse.tile as tile
from concourse import mybir
from concourse.bass_utils import run_bass_kernel_spmd

B, S, T, H, D = 4, 128, 128, 256, 3
P = 128          # partitions
K = H // P       # 2 k-tiles of H on partitions
F32 = mybir.dt.float32
F16 = mybir.dt.float16
TANH = mybir.ActivationFunctionType.Tanh

# blob column layout (fp32 words per partition)
# weights (fp16 via bitcast): 5 matrices (whh0, wih1, whh1, wih2, whh2)
NW = 5
WCW = NW * K * H // 2        # fp32 words used by weights
BT4 = WCW                    # biasT4 lhsT [4p, 128] fp16 -> 64 f32 cols
IND4 = BT4 + 64              # ind4 rhs [4p, 512] fp16 -> 256 f32 cols
P0 = IND4 + 256              # pre0 cols (fp32): (m, s) -> P0 + m*S + s
CW = P0 + K * S

_WSLOT = {(0, "hh"): 0, (1, "ih"): 1, (1, "hh"): 2, (2, "ih"): 3, (2, "hh"): 4}

OCHUNK = 32

_cache = {}


def _patched_drain_and_barrier(self, tick_clock, wait_clock):
    """Replacement for TileContext._drain_and_barrier.

    This walrus build lowers at most ONE sync-wait per instruction; the stock
    tail drain carries one wait per active proc.  Semantically the waits only
    need to complete before the final barrier's semaphore cleanup, so spread
    them over single-wait NOPs on the sync engine after the drain.
    """
    drain_inst = self.nc.sync.drain()
    wait_clock.add_sem_waits(
        drain_inst.ins, tile.ScopedClock({None: tick_clock.global_clock})
    )
    ins = drain_inst.ins
    si = ins.sync_info
    if si is not None and len(si.on_wait) > 1:
        waits = list(si.on_wait)
        ins.sync_info = mybir.SyncInfo(on_wait=[waits[0]],
                                       on_update=list(si.on_update))
        for w in waits[1:]:
            nop = self.nc.sync.nop(nofuse=True)
            nop.ins.sync_info = mybir.SyncInfo(on_wait=[w], on_update=[])

    self.nc.all_engine_barrier()
    assert self.sems is not None
    popped = self.nc._tile_sem_poison_stack.pop()
    assert popped is self._sem_poison
    self.nc.clear_and_free_semaphores(list(self.sems.allocated().values()))
    self.nc.all_engine_barrier()


tile.TileContext._drain_and_barrier = _patched_drain_and_barrier


def _build():
    nc = bass.Bass(trn_type="TRN2")

    blob = nc.dram_tensor("blob", [P, CW], F32, kind="ExternalInput")
    # DRAM layout mirrors the SBUF d2 plane ([p, s, k, v]); host reassembles
    # H = k*128+p.
    out = nc.dram_tensor("out", [P, S, K, T], F16, kind="ExternalOutput")
    out_c = out[:, :, :, :]

    with tile.TileContext(nc) as tc:
        with (
            tc.tile_pool(name="consts", bufs=1) as consts,
            tc.tile_pool(name="ps", bufs=2, space="PSUM") as psp,
            tc.tile_pool(name="psi", bufs=1, space="PSUM") as psip,
        ):
            cb = consts.tile([P, CW], F32)
            nc.gpsimd.dma_start(out=cb, in_=blob[:, :])
            cb16 = cb[:, 0:WCW].bitcast(F16)
            bt4 = cb[:, BT4:BT4 + 64].bitcast(F16)     # [128p, 128] (rows 0-3 used)
            ind4 = cb[:, IND4:IND4 + 256].bitcast(F16)  # [128p, 512] (rows 0-3 used)

            def w16(slot, k, m):
                c = (slot * K + k) * H + m * P
                return cb16[:, c:c + P]

            def wih(d, k, m):
                return w16(_WSLOT[(d, "ih")], k, m)

            def whh(d, k, m):
                return w16(_WSLOT[(d, "hh")], k, m)

            def pre0(m, s):
                c = P0 + m * S + s
                return cb[:, c:c + 1]

            zeros = consts.tile([P, K, T], F16)
            nc.vector.memset(zeros, 0.0)
            # ScalarE absorber: folds the blob-DMA semaphore into ACT's clock
            scr = consts.tile([P, 4], F32)
            nc.scalar.copy(out=scr[:, 0:1], in_=pre0(0, 0))
            # PE absorber + warmup: folds the blob-DMA semaphore into PE's clock
            dummy = psip.tile([32, 32], F32, tag="init")
            nc.tensor.matmul(dummy[:, :], lhsT=cb16[0:32, 0:32], rhs=cb16[0:32, 0:32],
                             start=True, stop=True)

            # U12 slot s: [0] = d2 step s-1, [1] = d1 step s  (so the fused
            # d2(t-2)/d1(t-1) tanh at tick t writes slot t-1 contiguously)
            u12 = consts.tile([P, S + 1, 2, K, T], F16)
            u0ring = consts.tile([P, 4, K, T], F16)

            for t in range(S + 2):
                d0_on = t < S
                d1_on = 1 <= t <= S
                d2_on = 2 <= t
                ps = psp.tile([P, 6, T], F32, tag="ps")

                # rank-4 bias fill for d2 ([0:256]) + d1 ([256:512])
                if d1_on or d2_on:
                    nc.tensor.matmul(ps[:, 0:4, :], lhsT=bt4[0:4, :],
                                     rhs=ind4[0:4, :], start=True, stop=False,
                                     skip_group_check=True)

                if d2_on:
                    s = t - 2
                    u_in = u12[:, s, 1, :, :]          # u1(s)
                    u_pr = zeros if s == 0 else u12[:, s, 0, :, :]  # u2(s-1)
                    for m in range(K):
                        first = False
                        for k in range(K):
                            nc.tensor.matmul(ps[:, m, :], lhsT=wih(2, k, m),
                                             rhs=u_in[:, k, :], start=False,
                                             stop=False, skip_group_check=True)
                        for k in range(K):
                            nc.tensor.matmul(ps[:, m, :], lhsT=whh(2, k, m),
                                             rhs=u_pr[:, k, :], start=False,
                                             stop=(k == K - 1),
                                             skip_group_check=True)
                if d1_on:
                    s = t - 1
                    u_in = u0ring[:, s % 4, :, :]      # u0(s)
                    u_pr = zeros if s == 0 else u12[:, s - 1, 1, :, :]  # u1(s-1)
                    for m in range(K):
                        for k in range(K):
                            nc.tensor.matmul(ps[:, 2 + m, :], lhsT=wih(1, k, m),
                                             rhs=u_in[:, k, :], start=False,
                                             stop=False, skip_group_check=True)
                        for k in range(K):
                            nc.tensor.matmul(ps[:, 2 + m, :], lhsT=whh(1, k, m),
                                             rhs=u_pr[:, k, :], start=False,
                                             stop=(k == K - 1),
                                             skip_group_check=True)
                if d0_on:
                    s = t
                    u_pr = zeros if s == 0 else u0ring[:, (s - 1) % 4, :, :]
                    for m in range(K):
                        for k in range(K):
                            nc.tensor.matmul(ps[:, 4 + m, :], lhsT=whh(0, k, m),
                                             rhs=u_pr[:, k, :], start=(k == 0),
                                             stop=(k == K - 1))

                # fused tanh for d2(t-2) + d1(t-1) -> u12 slot t-1
                if d1_on and d2_on:
                    nc.scalar.activation(u12[:, t - 1, :, :, :], ps[:, 0:4, :],
                                         TANH, bias=0.0)
                elif d1_on:      # t == 1
                    nc.scalar.activation(u12[:, t - 1, 1, :, :], ps[:, 2:4, :],
                                         TANH, bias=0.0)
                elif d2_on:      # t == S+1
                    nc.scalar.activation(u12[:, t - 1, 0, :, :], ps[:, 0:2, :],
                                         TANH, bias=0.0)
                if d0_on:
                    for m in range(K):
                        nc.scalar.activation(u0ring[:, t % 4, m, :],
                                             ps[:, 4 + m, :], TANH,
                                             bias=pre0(m, t))

                # d2 chunk [s0, s0+OCHUNK) complete once step s0+OCHUNK-1
                # (slot s0+OCHUNK) was written at t = s0+OCHUNK+1
                if d2_on and (t - 1) % OCHUNK == 0:
                    s0 = t - 1 - OCHUNK
                    nc.gpsimd.dma_start(
                        out=out_c[:, s0:s0 + OCHUNK, :, :],
                        in_=u12[:, s0 + 1:s0 + 1 + OCHUNK, 0, :, :])

    return nc


def _blob(seed, wT_ih, wT_hh, bih, bhh):
    """Pack per-core constants into the [P, CW] blob.

    seed: [S, H] fp32; wT_ih/wT_hh: [D, H, H] (W[d].T); biases [D, H].
    """
    b = np.zeros((P, CW), np.float32)
    w = np.empty((NW, H, H), np.float32)
    w[0] = wT_hh[0]
    w[1], w[2] = wT_ih[1], wT_hh[1]
    w[3], w[4] = wT_ih[2], wT_hh[2]
    w16 = (w.reshape(NW, K, P, H).transpose(2, 0, 1, 3)
           .reshape(P, NW * K * H).astype(np.float16))
    b[:, 0:WCW] = w16.view(np.float32)
    bs = (bih + bhh).astype(np.float32)
    # biasT4 rows: (b2m0, b2m1, b1m0, b1m1) on partitions 0..3, 128 fp16 cols
    bt4 = np.zeros((P, 128), np.float16)
    bt4[0, :] = bs[2, 0:128]
    bt4[1, :] = bs[2, 128:256]
    bt4[2, :] = bs[1, 0:128]
    bt4[3, :] = bs[1, 128:256]
    b[:, BT4:BT4 + 64] = bt4.view(np.float32)
    # ind4: block identity [4, 512]
    i4 = np.zeros((P, 512), np.float16)
    for r in range(4):
        i4[r, r * 128:(r + 1) * 128] = 1.0
    b[:, IND4:IND4 + 256] = i4.view(np.float32)
    # pre0[s] = seed[s] @ W_ih[0] (wT_ih[0] is already W.T) + bias0
    p0 = seed.astype(np.float32) @ wT_ih[0].astype(np.float32) + bs[0]
    b[:, P0:P0 + K * S] = p0.reshape(S, K, P).transpose(2, 1, 0).reshape(P, K * S)
    return b


def kernel(src, trg, Wx_ih, Wx_hh, bx_ih, bx_hh, Wy_ih, Wy_hh, by_ih, by_hh):
    if "nc" not in _cache:
        _cache["nc"] = _build()
    nc = _cache["nc"]

    def tr(w):  # [D,H,H] -> W[d].T contiguous
        return np.ascontiguousarray(np.swapaxes(np.asarray(w, np.float32), 1, 2))

    src = np.asarray(src, np.float32)
    trg = np.asarray(trg, np.float32)
    wx_ihT, wx_hhT = tr(Wx_ih), tr(Wx_hh)
    wy_ihT, wy_hhT = tr(Wy_ih), tr(Wy_hh)
    bx_ih = np.asarray(bx_ih, np.float32)
    bx_hh = np.asarray(bx_hh, np.float32)
    by_ih = np.asarray(by_ih, np.float32)
    by_hh = np.asarray(by_hh, np.float32)

    in_maps = []
    for b in range(B):  # cores 0-3: x chains
        in_maps.append({"blob": _blob(src[b], wx_ihT, wx_hhT, bx_ih, bx_hh)})
    for b in range(B):  # cores 4-7: y chains
        in_maps.append({"blob": _blob(trg[b], wy_ihT, wy_hhT, by_ih, by_hh)})

    _cache["last_in_maps"] = in_maps
    globals()["_last_in_maps"] = in_maps
    res = run_bass_kernel_spmd(nc, in_maps, list(range(8)))

    out = np.empty((B, S, T, 2, H), np.float32)
    ii = np.arange(S)[:, None]
    jj = np.arange(T)[None, :]
    idx = (jj - ii) % T  # hx[i,j] = u_i[(j-i)%T]
    for b in range(B):
        # raw core output [p, s, k, v] -> [s, H=k*128+p, v]
        arr = (res.results[b]["out"].astype(np.float32)
               .transpose(1, 2, 0, 3).reshape(S, H, T))
        hx = np.take_along_axis(arr, idx[:, None, :], axis=2)  # [s, H, j]
        out[b, :, :, 0, :] = hx.transpose(0, 2, 1)
        arr = (res.results[B + b]["out"].astype(np.float32)
               .transpose(1, 2, 0, 3).reshape(S, H, T))
        out[b, :, :, 1, :] = arr.transpose(2, 0, 1)  # [j, H, i] -> [i, j, H]
    return out


# revision 13
# speedup vs baseline: 1.6559x; 1.6559x over previous
"""GridRNN Trainium2 kernel.

Problem: 2-D grid RNN, B=4, S=T=128, H=256, D=3 depths.
  hx[d][b,i,j] = tanh(xin @ Wx_ih[d].T + bx_ih[d] + hx[d][b,i-1,(j-1)%T] @ Wx_hh[d].T + bx_hh[d])
  hy[d][b,i,j] = tanh(yin @ Wy_ih[d].T + by_ih[d] + hy[d][b,i,j-1]     @ Wy_hh[d].T + by_hh[d])
  (xin/yin = src/trg broadcast at d=0, previous depth's hx/hy for d>0)
  out = stack([hx[D-1], hy[D-1]], axis=-2)   # [B,S,T,2,H]

Key structure: the x-chain and y-chain never mix across depths -> 8 cores =
4 batches x 2 chains.  The x-chain's diagonal dependence hx[i-1,(j-1)%T] is
removed by shearing: u_i[c] = hx[i,(i+c)%T] turns it into a plain carry
u_{i-1}[c], identical in form to the y-chain.  One SPMD program runs on all
8 cores; only the input data differs per core.  The host unshears the x
outputs and transposes the y outputs.

Matmuls run in fp16 (1 cycle/row on the PE vs 4 for fp32); PSUM accumulates
fp32 and tanh reads the fp32 PSUM, so only operand rounding (~5e-4) enters
per step.  Depth-0's input term depends only on the step index, so it is
precomputed on the HOST and folded into the d0 tanh bias columns.

HAM throttling dominates this kernel's performance: the PE runs at K=4/8
(half clock, ~107ns per 128-col matmul) unless kept continuously busy, and
needs ~14us of gapless matmuls to un-throttle to K=8/8 (~56ns).  Hence:
 - a warm-up train of dummy matmuls runs while the const blob DMA streams,
 - the wavefront is scheduled stall-free: each stage's tanh is issued
   immediately after its own matmuls, and d2 lags the wavefront by 4 ticks
   so all its inputs are >=2 ticks old.
ACT-instruction economy: d1/d2 biases are injected into PSUM by trailing
rank-2 matmuls (lhsT = 2 bias rows, rhs = block-identity), so each stage's
tanh is ONE fused [128,256] ACT (this walrus build allows only ONE sync-wait
per hardware instruction, so every PSUM byte an ACT reads must be written by
a single engine - the PE).  d0 keeps per-m ACTs with the pre0 column bias.
"""

import numpy as np

import concourse.bass as bass
import concourse.tile as tile
from concourse import mybir
from concourse.bass_utils import run_bass_kernel_spmd

B, S, T, H, D = 4, 128, 128, 256, 3
P = 128          # partitions
K = H // P       # 2 k-tiles of H on partitions
F32 = mybir.dt.float32
F16 = mybir.dt.float16
TANH = mybir.ActivationFunctionType.Tanh

LAG2 = 4         # d2 wavefront lag (d0: t, d1: t-1, d2: t-LAG2)
NWARM = 30       # warm-up dummy matmuls (N=512): ~4.3us cold + ~4.3us warm,
                 # covers the HAM SHORT window (~3.4us) plus blob-DMA latency

# blob column layout (fp32 words per partition)
# weights (fp16 via bitcast): 5 matrices (whh0, wih1, whh1, wih2, whh2)
NW = 5
WCW = NW * K * H // 2        # fp32 words used by weights
BT2 = WCW                    # bias rows [2p, 128] fp16 x2 stages -> 2*64 f32
IND2 = BT2 + 128             # ind2 rhs [2p, 256] fp16 -> 128 f32 cols
P0 = IND2 + 128              # pre0 cols (fp32): (m, s) -> P0 + m*S + s
CW = P0 + K * S

_WSLOT = {(0, "hh"): 0, (1, "ih"): 1, (1, "hh"): 2, (2, "ih"): 3, (2, "hh"): 4}

OCHUNK = 32

_cache = {}


def _patched_drain_and_barrier(self, tick_clock, wait_clock):
    """Replacement for TileContext._drain_and_barrier.

    This walrus build lowers at most ONE sync-wait per instruction; the stock
    tail drain carries one wait per active proc.  Semantically the waits only
    need to complete before the final barrier's semaphore cleanup, so spread
    them over single-wait NOPs on the sync engine after the drain.
    """
    drain_inst = self.nc.sync.drain()
    wait_clock.add_sem_waits(
        drain_inst.ins, tile.ScopedClock({None: tick_clock.global_clock})
    )
    ins = drain_inst.ins
    si = ins.sync_info
    if si is not None and len(si.on_wait) > 1:
        waits = list(si.on_wait)
        ins.sync_info = mybir.SyncInfo(on_wait=[waits[0]],
                                       on_update=list(si.on_update))
        for w in waits[1:]:
            nop = self.nc.sync.nop(nofuse=True)
            nop.ins.sync_info = mybir.SyncInfo(on_wait=[w], on_update=[])

    self.nc.all_engine_barrier()
    assert self.sems is not None
    popped = self.nc._tile_sem_poison_stack.pop()
    assert popped is self._sem_poison
    self.nc.clear_and_free_semaphores(list(self.sems.allocated().values()))
    self.nc.all_engine_barrier()


tile.TileContext._drain_and_barrier = _patched_drain_and_barrier


def _split_multi_waits(nc):
    """Move excess sync-waits onto the preceding same-engine instruction.

    This walrus build allows one sync-wait per hardware instruction.  The
    Tile scheduler occasionally leaves an instruction with two (e.g. a
    reordered matmul waiting on both the ACT sem and the PE sem).  The
    engine queue is in-order, so a wait carried by the immediately preceding
    same-engine instruction gates the original instruction identically.
    """
    fn = nc.m.functions[0]
    insts = []

    def walk(block):
        for ins in block.instructions:
            insts.append(ins)
            for b in getattr(ins, "blocks", []) or []:
                walk(b)

    for bb in fn.blocks:
        walk(bb)

    prev_by_engine = {}
    for ins in insts:
        eng = ins.engine
        si = ins.sync_info
        if si is not None and len(si.on_wait) > 1:
            waits = list(si.on_wait)
            while len(waits) > 1:
                carrier = prev_by_engine.get(eng)
                assert carrier is not None, "no wait carrier available"
                csi = carrier.sync_info
                assert csi is None or not csi.on_wait, (
                    f"carrier {carrier.name} already has waits")
                w = waits.pop(0)
                carrier.sync_info = mybir.SyncInfo(
                    on_wait=[w],
                    on_update=list(csi.on_update) if csi else [])
                # carrier now has a wait; it can't carry more
                prev_by_engine[eng] = None
            ins.sync_info = mybir.SyncInfo(on_wait=waits,
                                           on_update=list(si.on_update))
        si = ins.sync_info
        prev_by_engine[eng] = ins if (si is None or not si.on_wait) else None


def _build():
    nc = bass.Bass(trn_type="TRN2")

    blob = nc.dram_tensor("blob", [P, CW], F32, kind="ExternalInput")
    # DRAM layout mirrors the SBUF d2 plane ([p, s, k, v]); host reassembles
    # H = k*128+p.
    out = nc.dram_tensor("out", [P, S, K, T], F16, kind="ExternalOutput")
    out_c = out[:, :, :, :]

    with tile.TileContext(nc) as tc:
        with (
            tc.tile_pool(name="consts", bufs=1) as consts,
            tc.tile_pool(name="ps0", bufs=2, space="PSUM") as ps0p,
            tc.tile_pool(name="ps1", bufs=2, space="PSUM") as ps1p,
            tc.tile_pool(name="ps2", bufs=2, space="PSUM") as ps2p,
            tc.tile_pool(name="psi", bufs=1, space="PSUM") as psip,
        ):
            # garbage warm-up operands: uninitialized SBUF, discarded PSUM.
            # Keeps the PE gapless while the blob DMA streams so HAM lifts
            # the clock to K=8/8 before tick 0.
            warm = consts.tile([P, 512], F16)
            wps = psip.tile([P, 512], F32, tag="warm")
            nc.vector.memset(warm, 0.0)
            for i in range(NWARM):
                nc.tensor.matmul(wps[:, :], lhsT=warm[:, 0:P], rhs=warm[:, :],
                                 start=True, stop=True, skip_group_check=True)

            cb = consts.tile([P, CW], F32)
            nc.gpsimd.dma_start(out=cb, in_=blob[:, :])
            cb16 = cb[:, 0:WCW].bitcast(F16)
            bt2 = cb[:, BT2:BT2 + 128].bitcast(F16)    # [p, 256]: d1 rows, d2 rows
            ind2 = cb[:, IND2:IND2 + 128].bitcast(F16)  # [p, 256] (rows 0-1 used)

            def w16(slot, k, m):
                c = (slot * K + k) * H + m * P
                return cb16[:, c:c + P]

            def wih(d, k, m):
                return w16(_WSLOT[(d, "ih")], k, m)

            def whh(d, k, m):
                return w16(_WSLOT[(d, "hh")], k, m)

            def pre0(m, s):
                c = P0 + m * S + s
                return cb[:, c:c + 1]

            zeros = consts.tile([P, K, T], F16)
            nc.vector.memset(zeros, 0.0)
            # ScalarE absorber: folds the blob-DMA semaphore into ACT's clock
            scr = consts.tile([P, 4], F32)
            nc.scalar.copy(out=scr[:, 0:1], in_=pre0(0, 0))
            # PE absorber: folds the blob-DMA semaphore into PE's clock
            nc.tensor.matmul(wps[0:32, 0:32], lhsT=cb16[0:32, 0:32],
                             rhs=cb16[0:32, 0:32], start=True, stop=True,
                             skip_group_check=True)

            u0ring = consts.tile([P, 4, K, T], F16)
            u1lin = consts.tile([P, S, K, T], F16)
            u2lin = consts.tile([P, S, K, T], F16)

            def stage_mms(ps, d, u_in, u_pr, bias_rows):
                """8 main MMs (4 if u_in None) + trailing rank-2 bias MM."""
                for m in range(K):
                    first = True
                    if u_in is not None:
                        for k in range(K):
                            nc.tensor.matmul(ps[:, m, :], lhsT=wih(d, k, m),
                                             rhs=u_in[:, k, :], start=first,
                                             stop=False)
                            first = False
                    for k in range(K):
                        nc.tensor.matmul(ps[:, m, :], lhsT=whh(d, k, m),
                                         rhs=u_pr[:, k, :], start=first,
                                         stop=False, skip_group_check=True)
                        first = False
                if bias_rows is not None:
                    nc.tensor.matmul(ps[:, :, :], lhsT=bias_rows,
                                     rhs=ind2[0:2, :], start=False, stop=True,
                                     skip_group_check=True)

            NT = S + LAG2
            for t in range(NT):
                d0_on = t < S
                d1_on = 1 <= t <= S
                d2_on = LAG2 <= t

                # ---- d0: step t (hidden only; input term lives in pre0 bias)
                if d0_on:
                    s = t
                    ps0 = ps0p.tile([P, K, T], F32, tag="ps0")
                    u_pr = zeros if s == 0 else u0ring[:, (s - 1) % 4, :, :]
                    stage_mms(ps0, 0, None, u_pr, None)
                    for m in range(K):
                        nc.scalar.activation(u0ring[:, s % 4, m, :],
                                             ps0[:, m, :], TANH,
                                             bias=pre0(m, s))
                # ---- d1: step t-1
                if d1_on:
                    s = t - 1
                    ps1 = ps1p.tile([P, K, T], F32, tag="ps1")
                    u_in = u0ring[:, s % 4, :, :]
                    u_pr = zeros if s == 0 else u1lin[:, s - 1, :, :]
                    stage_mms(ps1, 1, u_in, u_pr, bt2[0:2, 0:128])
                    nc.scalar.activation(u1lin[:, s, :, :], ps1[:, :, :],
                                         TANH, bias=0.0)
                # ---- d2: step t-LAG2
                if d2_on:
                    s = t - LAG2
                    ps2 = ps2p.tile([P, K, T], F32, tag="ps2")
                    u_in = u1lin[:, s, :, :]
                    u_pr = zeros if s == 0 else u2lin[:, s - 1, :, :]
                    stage_mms(ps2, 2, u_in, u_pr, bt2[0:2, 128:256])
                    nc.scalar.activation(u2lin[:, s, :, :], ps2[:, :, :],
                                         TANH, bias=0.0)
                    if (s + 1) % OCHUNK == 0:
                        s0 = s + 1 - OCHUNK
                        nc.gpsimd.dma_start(
                            out=out_c[:, s0:s0 + OCHUNK, :, :],
                            in_=u2lin[:, s0:s0 + OCHUNK, :, :])

    _split_multi_waits(nc)
    return nc


def _blob(seed, wT_ih, wT_hh, bih, bhh):
    """Pack per-core constants into the [P, CW] blob.

    seed: [S, H] fp32; wT_ih/wT_hh: [D, H, H] (W[d].T); biases [D, H].
    """
    b = np.zeros((P, CW), np.float32)
    w = np.empty((NW, H, H), np.float32)
    w[0] = wT_hh[0]
    w[1], w[2] = wT_ih[1], wT_hh[1]
    w[3], w[4] = wT_ih[2], wT_hh[2]
    w16 = (w.reshape(NW, K, P, H).transpose(2, 0, 1, 3)
           .reshape(P, NW * K * H).astype(np.float16))
    b[:, 0:WCW] = w16.view(np.float32)
    bs = (bih + bhh).astype(np.float32)
    # bias rows: cols 0-127 = d1 (rows: b1m0, b1m1), cols 128-255 = d2
    bt2 = np.zeros((P, 256), np.float16)
    bt2[0, 0:128] = bs[1, 0:128]
    bt2[1, 0:128] = bs[1, 128:256]
    bt2[0, 128:256] = bs[2, 0:128]
    bt2[1, 128:256] = bs[2, 128:256]
    b[:, BT2:BT2 + 128] = bt2.view(np.float32)
    # ind2: block identity [2, 256]
    i2 = np.zeros((P, 256), np.float16)
    i2[0, 0:128] = 1.0
    i2[1, 128:256] = 1.0
    b[:, IND2:IND2 + 128] = i2.view(np.float32)
    # pre0[s] = seed[s] @ W_ih[0] (wT_ih[0] is already W.T) + bias0
    p0 = seed.astype(np.float32) @ wT_ih[0].astype(np.float32) + bs[0]
    b[:, P0:P0 + K * S] = p0.reshape(S, K, P).transpose(2, 1, 0).reshape(P, K * S)
    return b


def kernel(src, trg, Wx_ih, Wx_hh, bx_ih, bx_hh, Wy_ih, Wy_hh, by_ih, by_hh):
    if "nc" not in _cache:
        _cache["nc"] = _build()
    nc = _cache["nc"]

    def tr(w):  # [D,H,H] -> W[d].T contiguous
        return np.ascontiguousarray(np.swapaxes(np.asarray(w, np.float32), 1, 2))

    src = np.asarray(src, np.float32)
    trg = np.asarray(trg, np.float32)
    wx_ihT, wx_hhT = tr(Wx_ih), tr(Wx_hh)
    wy_ihT, wy_hhT = tr(Wy_ih), tr(Wy_hh)
    bx_ih = np.asarray(bx_ih, np.float32)
    bx_hh = np.asarray(bx_hh, np.float32)
    by_ih = np.asarray(by_ih, np.float32)
    by_hh = np.asarray(by_hh, np.float32)

    in_maps = []
    for b in range(B):  # cores 0-3: x chains
        in_maps.append({"blob": _blob(src[b], wx_ihT, wx_hhT, bx_ih, bx_hh)})
    for b in range(B):  # cores 4-7: y chains
        in_maps.append({"blob": _blob(trg[b], wy_ihT, wy_hhT, by_ih, by_hh)})

    _cache["last_in_maps"] = in_maps
    globals()["_last_in_maps"] = in_maps
    res = run_bass_kernel_spmd(nc, in_maps, list(range(8)))

    out = np.empty((B, S, T, 2, H), np.float32)
    ii = np.arange(S)[:, None]
    jj = np.arange(T)[None, :]
    idx = (jj - ii) % T  # hx[i,j] = u_i[(j-i)%T]
    for b in range(B):
        # raw core output [p, s, k, v] -> [s, H=k*128+p, v]
        arr = (res.results[b]["out"].astype(np.float32)
               .transpose(1, 2, 0, 3).reshape(S, H, T))
        hx = np.take_along_axis(arr, idx[:, None, :], axis=2)  # [s, H, j]
        out[b, :, :, 0, :] = hx.transpose(0, 2, 1)
        arr = (res.results[B + b]["out"].astype(np.float32)
               .transpose(1, 2, 0, 3).reshape(S, H, T))
        out[b, :, :, 1, :] = arr.transpose(2, 0, 1)  # [j, H, i] -> [i, j, H]
    return out
